# revision 12
# baseline (speedup 1.0000x reference)
"""Multi-head causal attention (B=2, S=2048, C=2048, H=16) on 8 NeuronCores.

Sharding: 2-way data parallel over batch x 4-way tensor parallel over heads.
Core i handles batch b = i // 4 and heads [4*(i%4), 4*(i%4)+4)  (local
channel slice m in [512*(i%4), 512*(i%4)+512)).

Per-core kernel:
  phase A: Qt/Kt (d on partitions) and V (s on partitions) projections in
           fp16 (fp32 PSUM), weights SBUF-cached, biases via pre-broadcast
           SBUF tiles
  phase B: causal attention per head: transposed score tiles (t on
           partitions, s free) in fp16, exp on ACT -> float32r, causal mask
           via gpsimd affine_select, AV + ones-matrix denominator matmuls
           (denominator lands replicated on all partitions), normalize via
           DVE reciprocal + multiply
  phase C: local output projection partial (s, j) in fp16 + bo/4; partials
           of the 4 cores in a batch group are summed on the host.
"""

import numpy as np

B, S, C, H = 2, 2048, 2048, 16
D = C // H            # 128 per-head dim
HL = 4                # heads per core
ML = HL * D           # 512 local channels
P = 128
SCALE = 1.0 / float(np.sqrt(D))

_CACHE = {}


def _build():
    import concourse.bacc as bacc
    import concourse.mybir as mybir
    import concourse.tile as tile

    f32 = mybir.dt.float32
    f32r = mybir.dt.float32r
    f16 = mybir.dt.float16
    Exp = mybir.ActivationFunctionType.Exp
    is_ge = mybir.AluOpType.is_ge
    add = mybir.AluOpType.add
    mult = mybir.AluOpType.mult

    nc = bacc.Bacc("TRN2", target_bir_lowering=False, debug=False, num_devices=8)

    xt = nc.dram_tensor("xt", [C, S], f16, kind="ExternalInput")       # x[b].T
    wqt = nc.dram_tensor("wqt", [C, ML], f16, kind="ExternalInput")    # Wq.T cols
    wkt = nc.dram_tensor("wkt", [C, ML], f16, kind="ExternalInput")
    wvt = nc.dram_tensor("wvt", [C, ML], f16, kind="ExternalInput")
    wot = nc.dram_tensor("wot", [ML, C], f16, kind="ExternalInput")    # Wo.T rows
    bq = nc.dram_tensor("bq", [ML], f32, kind="ExternalInput")
    bk = nc.dram_tensor("bk", [ML], f32, kind="ExternalInput")
    bv = nc.dram_tensor("bv", [ML], f32, kind="ExternalInput")
    bo4 = nc.dram_tensor("bo4", [C], f32, kind="ExternalInput")        # bo / 4
    ones2 = nc.dram_tensor("ones2", [P, P], f32, kind="ExternalInput")
    out = nc.dram_tensor("out", [S, C], f32, kind="ExternalOutput")

    NT = S // P    # 16 t/s tiles of 128
    NCT = C // P   # 16 contraction tiles

    with tile.TileContext(nc) as tc:
        with tc.tile_pool(name="persist", bufs=1) as pp_:
            Q = [pp_.tile([P, S], f16, tag=f"q{m}", name=f"q{m}") for m in range(HL)]
            K = [pp_.tile([P, S], f16, tag=f"k{m}", name=f"k{m}") for m in range(HL)]
            V = [pp_.tile([P, ML], f32r, tag=f"v{t}", name=f"v{t}") for t in range(NT)]
            WO = [pp_.tile([P, C], f16, tag=f"wo{m}", name=f"wo{m}") for m in range(HL)]
            ones_mat = pp_.tile([P, P], f32r, tag="ones_mat", name="ones_mat")
            bv_row = pp_.tile([1, ML], f32, tag="bv_row", name="bv_row")
            bo_row = pp_.tile([1, C], f32, tag="bo_row", name="bo_row")
            bv_bc = pp_.tile([P, ML], f32, tag="bv_bc", name="bv_bc")
            bo_bc = pp_.tile([P, C], f32, tag="bo_bc", name="bo_bc")
            bq_t = [pp_.tile([P, 1], f32, tag=f"bq{m}", name=f"bq{m}") for m in range(HL)]
            bk_t = [pp_.tile([P, 1], f32, tag=f"bk{m}", name=f"bk{m}") for m in range(HL)]

            # small bias loads on the gpsimd queue; bulk loads ordered by
            # first use on the sync queue (in-order issue per engine)
            nc.gpsimd.dma_start(bv_row[:], bv[None, :])
            nc.gpsimd.dma_start(bo_row[:], bo4[None, :])
            nc.gpsimd.dma_start(ones_mat[:], ones2[:, :].bitcast(f32r))
            for m in range(HL):
                nc.gpsimd.dma_start(bq_t[m][:], bq[m * P:(m + 1) * P, None])
                nc.gpsimd.dma_start(bk_t[m][:], bk[m * P:(m + 1) * P, None])
            nc.gpsimd.partition_broadcast(bv_bc[:], bv_row[:])
            nc.gpsimd.partition_broadcast(bo_bc[:], bo_row[:])

            # ---------------- phase A: projections ----------------
            with tc.tile_pool(name="workA", bufs=1) as wa, \
                 tc.tile_pool(name="psA", bufs=1, space="PSUM") as psa:
                # weight caches; wv first (first use), interleaved with sb0 x
                wq_c, wk_c, wv_c = [], [], []
                xt_t0 = []
                for c in range(NCT):
                    t = wa.tile([P, ML], f16, tag=f"cv{c}", name=f"cv{c}")
                    nc.sync.dma_start(t[:], wvt[c * P:(c + 1) * P, :])
                    wv_c.append(t)
                    xti = wa.tile([P, 512], f16, tag=f"xt{c}", bufs=2,
                                  name=f"xt{c}_0")
                    nc.sync.dma_start(xti[:], xt[c * P:(c + 1) * P, 0:512])
                    xt_t0.append(xti)
                for c in range(NCT):
                    t = wa.tile([P, ML], f16, tag=f"cq{c}", name=f"cq{c}")
                    nc.sync.dma_start(t[:], wqt[c * P:(c + 1) * P, :])
                    wq_c.append(t)
                for c in range(NCT):
                    t = wa.tile([P, ML], f16, tag=f"ck{c}", name=f"ck{c}")
                    nc.sync.dma_start(t[:], wkt[c * P:(c + 1) * P, :])
                    wk_c.append(t)
                for m in range(HL):
                    nc.gpsimd.dma_start(WO[m][:], wot[m * P:(m + 1) * P, :])

                for sb in range(4):  # 512-wide s blocks
                    s0 = sb * 512
                    if sb == 0:
                        xt_t = xt_t0
                    else:
                        xt_t = []
                        for c in range(NCT):
                            xti = wa.tile([P, 512], f16, tag=f"xt{c}", bufs=2,
                                          name=f"xt{c}_{sb}")
                            nc.sync.dma_start(
                                xti[:], xt[c * P:(c + 1) * P, s0:s0 + 512])
                            xt_t.append(xti)

                    # V projection (s on partitions)
                    pv = [psa.tile([P, ML], f32, tag="acc", bufs=8,
                                   name=f"pv{i}_{sb}") for i in range(4)]
                    for c in range(NCT):
                        for i in range(4):
                            nc.tensor.matmul(
                                pv[i][:], xt_t[c][:, i * P:(i + 1) * P],
                                wv_c[c][:],
                                start=(c == 0), stop=(c == NCT - 1))
                    for i in range(4):
                        nc.vector.tensor_add(V[sb * 4 + i][:], pv[i][:], bv_bc[:])

                    # Q then K projections (d on partitions)
                    for proj in range(2):
                        w_c = wq_c if proj == 0 else wk_c
                        pa = [psa.tile([P, 512], f32, tag="acc", bufs=8,
                                       name=f"pa{proj}{i}_{sb}") for i in range(4)]
                        for c in range(NCT):
                            for m in range(HL):
                                nc.tensor.matmul(
                                    pa[m][:], w_c[c][:, m * P:(m + 1) * P],
                                    xt_t[c][:],
                                    start=(c == 0), stop=(c == NCT - 1))
                        for m in range(HL):
                            if proj == 0:
                                nc.vector.tensor_scalar(
                                    Q[m][:, s0:s0 + 512], pa[m][:],
                                    bq_t[m][:], SCALE, add, mult)
                            else:
                                nc.vector.tensor_scalar_add(
                                    K[m][:, s0:s0 + 512], pa[m][:], bk_t[m][:])

            # ---------------- phases B + C per 1024-s-block ----------------
            SB = 1024
            with tc.tile_pool(name="workBC", bufs=1) as wb, \
                 tc.tile_pool(name="psBC", bufs=1, space="PSUM") as psb:
                for sb in range(S // SB):
                    s0 = sb * SB
                    ntile = (SB // P) * (sb + 1)
                    ot = []
                    for h in range(HL):
                        po = psb.tile([P, SB], f32, tag="po", bufs=1,
                                      name=f"po{sb}{h}")
                        pd = psb.tile([P, SB], f32, tag="pd", bufs=1,
                                      name=f"pd{sb}{h}")
                        for ti in range(ntile):
                            # diagonal tiles: scores only needed for
                            # s >= ti*128 -> trim the free range to [off, SB)
                            off = max(0, ti * P - s0)
                            W = SB - off
                            ps = psb.tile([P, SB], f32, tag="mm", bufs=2,
                                          name=f"ps{sb}{h}{ti}")
                            # chunks must not cross PSUM bank boundaries
                            pos = off
                            while pos < SB:
                                w = min(512 - (pos % 512), SB - pos)
                                nc.tensor.matmul(
                                    ps[:, pos:pos + w],
                                    K[h][:, ti * P:(ti + 1) * P],
                                    Q[h][:, s0 + pos:s0 + pos + w],
                                    start=True, stop=True)
                                pos += w
                            pe = wb.tile([P, SB], f32r, tag="pe", bufs=3,
                                         name=f"pe{sb}{h}{ti}")
                            nc.scalar.activation(pe[:, off:SB], ps[:, off:SB],
                                                 Exp)
                            if off or ti == (SB // P) * sb:  # diagonal: mask
                                nc.gpsimd.affine_select(
                                    out=pe[:, off:SB], in_=pe[:, off:SB],
                                    compare_op=is_ge,
                                    fill=0.0, base=0,
                                    pattern=[[1, W]], channel_multiplier=-1)
                            last = ti == ntile - 1
                            pos = off
                            while pos < SB:
                                w = min(512 - (pos % 512), SB - pos)
                                nc.tensor.matmul(pd[:, pos:pos + w],
                                                 ones_mat[:],
                                                 pe[:, pos:pos + w],
                                                 start=(ti == 0), stop=last)
                                nc.tensor.matmul(po[:, pos:pos + w],
                                                 V[ti][:, h * P:(h + 1) * P],
                                                 pe[:, pos:pos + w],
                                                 start=(ti == 0), stop=last)
                                pos += w
                        rec = wb.tile([P, SB], f32, tag="rec", bufs=2,
                                      name=f"rec{sb}{h}")
                        nc.vector.reciprocal(rec[:], pd[:])
                        oth = wb.tile([P, SB], f16, tag=f"ot{h}", bufs=2,
                                      name=f"ot{sb}{h}")
                        nc.vector.tensor_mul(oth[:], po[:], rec[:])
                        ot.append(oth)

                    # phase C: output projection partial for this s block
                    for jb in range(2):
                        j0 = jb * SB
                        for st in range(SB // P):
                            pp = psb.tile([P, SB], f32, tag="mm", bufs=2,
                                          name=f"pp{sb}{jb}{st}")
                            for m in range(HL):
                                for jx in range(2):
                                    nc.tensor.matmul(
                                        pp[:, jx * 512:(jx + 1) * 512],
                                        ot[m][:, st * P:(st + 1) * P],
                                        WO[m][:, j0 + jx * 512:j0 + (jx + 1) * 512],
                                        start=(m == 0), stop=(m == HL - 1))
                            outt = wb.tile([P, SB], f32, tag="outt", bufs=3,
                                           name=f"outt{sb}{jb}{st}")
                            nc.vector.tensor_add(outt[:], pp[:],
                                                 bo_bc[:, j0:j0 + SB])
                            nc.sync.dma_start(
                                out[s0 + st * P:s0 + (st + 1) * P, j0:j0 + SB],
                                outt[:])

    nc.compile()
    return nc


def _get_program():
    if "nc" not in _CACHE:
        _CACHE["nc"] = _build()
    return _CACHE["nc"]


def make_in_maps(x, Wq, bq, Wk, bk, Wv, bv, Wo, bo):
    xtb = [np.ascontiguousarray(x[b].T).astype(np.float16) for b in range(B)]
    WqT = np.ascontiguousarray(Wq.T).astype(np.float16)
    WkT = np.ascontiguousarray(Wk.T).astype(np.float16)
    WvT = np.ascontiguousarray(Wv.T).astype(np.float16)
    WoT = np.ascontiguousarray(Wo.T).astype(np.float16)
    ones2 = np.ones((P, P), dtype=np.float32)
    bo4 = (bo * 0.25).astype(np.float32)
    in_maps = []
    for core in range(8):
        b, hg = divmod(core, 4)
        ms = slice(hg * ML, (hg + 1) * ML)
        in_maps.append({
            "xt": xtb[b],
            "wqt": np.ascontiguousarray(WqT[:, ms]),
            "wkt": np.ascontiguousarray(WkT[:, ms]),
            "wvt": np.ascontiguousarray(WvT[:, ms]),
            "wot": np.ascontiguousarray(WoT[ms, :]),
            "bq": np.ascontiguousarray(bq[ms]),
            "bk": np.ascontiguousarray(bk[ms]),
            "bv": np.ascontiguousarray(bv[ms]),
            "bo4": bo4,
            "ones2": ones2,
        })
    return in_maps


def run(inputs, trace=False):
    from concourse.bass_utils import run_bass_kernel_spmd

    nc = _get_program()
    in_maps = make_in_maps(
        inputs["x"], inputs["Wq"], inputs["bq"], inputs["Wk"], inputs["bk"],
        inputs["Wv"], inputs["bv"], inputs["Wo"], inputs["bo"])
    res = run_bass_kernel_spmd(nc, in_maps, core_ids=list(range(8)), trace=trace)
    partials = [np.asarray(res.results[c]["out"]) for c in range(8)]
    full = np.empty((B, S, C), dtype=np.float32)
    for b in range(B):
        acc = np.sum(np.stack(partials[4 * b:4 * b + 4], 0), 0,
                     dtype=np.float64)
        full[b] = acc.astype(np.float32)
    return full, res


def kernel(**inputs):
    full, _ = run(inputs, trace=False)
    return full
